# revision 26
# baseline (speedup 1.0000x reference)
"""Two-layer GAT on 8 Trainium2 NeuronCores.

Strategy (dst-partitioned edge parallelism, degree-sorted blocks):
  - Core c owns nodes [c*SH, (c+1)*SH) for the feature matmul and as edge
    destinations, so the segment softmax over incoming edges is core-local.
  - Per core, dst nodes are in-degree sorted into blocks of 128 (one node
    per SBUF partition); a node's incoming edges lie along the free dim.
  - Edge gathers use nc.gpsimd.dma_gather (int16 indices). The gather
    table packs 4 nodes per row (row = gpos//4, class = gpos%4) so row ids
    fit in int16; each class is a strided column slice of the table.
    Edge slots are therefore grouped per (block, class-of-src) segment,
    padded to the cross-core max; pad slots gather a sentinel unit whose
    alpha_l = -1000 so exp() -> 0.
  - Layer-1 units are [xl f16 x128 | alpha_l f32 | pad] (512B); layer-2
    units are [h2 f32 x40 | alpha_l2 f32 | pad] (256B). alpha_r is a
    per-partition ACT bias; denominators come from the ACT Exp accumulator;
    the division is hoisted out of the edge sum.
  - Blocks are processed in groups; within a group the grid is class-major
    so one dma_gather window covers many blocks. Per-(block,class) partial
    sums accumulate into SBUF accumulator tiles.
  - The layer-2 projection (W2, att vectors) is fused into the layer-1
    block epilogue (PE transpose + matmul); an 8-core AllGather exchanges
    the packed tables between layers.
"""

import sys

for _p in ("/opt/trn_rl_repo",):
    if _p not in sys.path:
        sys.path.insert(0, _p)

import numpy as np

try:
    import tempfile

    import jax

    _cc_dir = tempfile.gettempdir() + "/jax_cc_cache"
    jax.config.update("jax_compilation_cache_dir", _cc_dir)
    jax.config.update("jax_persistent_cache_min_entry_size_bytes", -1)
    jax.config.update("jax_persistent_cache_min_compile_time_secs", 0)
except Exception:
    pass

N_CORES = 8
P = 128
GB = 33        # blocks per sweep group
WCOLS = 64     # max gather-window width in slot-columns (128 edges each)
SENT_AL = -1000.0
S_OUT = 512.0  # int16 output scale


# ---------------------------------------------------------------- host prep
def _host_prep(x, edge_index, W1, att_l1, att_r1, b1, W2, att_l2, att_r2, b2):
    x = np.asarray(x, np.float32)
    ei = np.asarray(edge_index).astype(np.int64)
    W1 = np.asarray(W1, np.float32)
    W2 = np.asarray(W2, np.float32)
    att_l1 = np.asarray(att_l1, np.float32)
    att_r1 = np.asarray(att_r1, np.float32)
    att_l2 = np.asarray(att_l2, np.float32)
    att_r2 = np.asarray(att_r2, np.float32)
    b1 = np.asarray(b1, np.float32)
    b2 = np.asarray(b2, np.float32)

    N, IN_C = x.shape
    HID = W1.shape[0]
    OUT_C = W2.shape[0]
    assert N % (N_CORES * 4) == 0
    SH = N // N_CORES
    NBLK = -(-SH // P)
    NROWS = N // 4  # packed table rows
    src, dst = ei[0], ei[1]
    owner = dst // SH

    perms = []
    for c in range(N_CORES):
        m = owner == c
        d0 = dst[m] - c * SH
        deg = np.bincount(d0, minlength=SH)
        perms.append(np.argsort(deg, kind="stable"))

    # --- class balancing: greedily pick each node's table class (pos%4)
    # to flatten per-(dst,class) incoming-edge counts, shrinking the
    # per-(block,class) max widths (= gather padding). Blocks keep their
    # degree-sorted membership; only the within-block lane order changes.
    pos = np.empty(N, np.int64)
    for c in range(N_CORES):
        pos[c * SH + perms[c]] = np.arange(SH)
    outdeg = np.bincount(src, minlength=N)
    indeg = np.bincount(dst, minlength=N)
    fair = np.ceil(indeg / 4.0).astype(np.int32)
    cnt4 = np.zeros((N, 4), np.int32)
    order_e = np.argsort(src, kind="stable")
    ed = dst[order_e]
    starts = np.searchsorted(src[order_e], np.arange(N))
    ends = np.searchsorted(src[order_e], np.arange(N) + 1)
    blk_key = (np.arange(N) // SH) * NBLK + (pos % SH) // P
    bsz = np.bincount(blk_key, minlength=N_CORES * NBLK)
    cap = np.repeat(bsz // 4, 4).reshape(-1, 4).astype(np.int64).copy()
    cls = np.empty(N, np.int64)
    for u in np.argsort(-outdeg, kind="stable"):
        e0, e1 = starts[u], ends[u]
        bk = blk_key[u]
        if e1 > e0:
            vs = ed[e0:e1]
            c4 = cnt4[vs]
            over = np.maximum(c4 + 1 - fair[vs][:, None], 0).sum(axis=0)
            cost = over * 1000.0 + c4.sum(axis=0)
        else:
            cost = np.zeros(4)
        cost = np.where(cap[bk] > 0, cost, np.inf)
        mm = int(np.argmin(cost))
        cls[u] = mm
        cap[bk, mm] -= 1
        if e1 > e0:
            cnt4[vs, mm] += 1
    # rebuild perms: within each block, lane l holds a class l%4 node
    invperms = []
    for c in range(N_CORES):
        newp = np.empty(SH, np.int64)
        cl_loc = cls[c * SH + perms[c]]  # class per old position
        for b in range(NBLK):
            lo = b * P
            hi = min(lo + P, SH)
            blknodes = perms[c][lo:hi]
            blkcls = cl_loc[lo:hi]
            nb = hi - lo
            for mm in range(4):
                sel = blknodes[blkcls == mm]
                newp[lo + mm : lo + nb : 4] = sel
        perms[c] = newp
        inv = np.empty(SH, np.int64)
        inv[newp] = np.arange(SH)
        invperms.append(inv)

    gpos = np.empty(N, np.int64)
    for c in range(N_CORES):
        gpos[c * SH + perms[c]] = c * SH + np.arange(SH)

    # per (block, class) widths, common max across cores
    Wbm = np.zeros((NBLK, 4), np.int64)
    per_core = []
    for c in range(N_CORES):
        m = owner == c
        s_c = src[m]
        d0 = dst[m] - c * SH
        pos = invperms[c][d0]         # dst slot position (block*128+lane)
        g = gpos[s_c]                 # src table position
        cls = (g % 4).astype(np.int64)
        row = g // 4
        blk = pos // P
        lane = pos % P
        cnt = np.zeros((NBLK, 4, P), np.int64)
        np.add.at(cnt, (blk, cls, lane), 1)
        Wbm = np.maximum(Wbm, cnt.max(axis=2))
        per_core.append((row, cls, blk, lane))

    # grid: groups of GB blocks, class-major inside the group
    colstart = np.zeros((NBLK, 4), np.int64)
    windows = []  # (colstart_global, ncols, class, group) per gather call
    col = 0
    b0 = 0
    grp = 0
    while b0 < NBLK:
        b1_ = min(b0 + GB, NBLK)
        for m in range(4):
            wstart = col
            wcols = 0
            for b in range(b0, b1_):
                w = int(Wbm[b, m])
                if wcols + w > WCOLS and wcols > 0:
                    windows.append((wstart, wcols, m, grp))
                    wstart = col
                    wcols = 0
                colstart[b, m] = col
                col += w
                wcols += w
            if wcols > 0:
                windows.append((wstart, wcols, m, grp))
        b0 = b1_
        grp += 1
    totcols = int(col)
    tot_slots = totcols * P
    tot_slots16 = -(-tot_slots // 16) * 16

    # int8-quantize x with a global scale folded into W1 (xl = x_q @ (s*W1).T)
    xscale = max(float(np.abs(x).max()), 1e-8) / 127.0
    w1s = (W1 * xscale).astype(np.float32)
    w1a = np.concatenate(
        [w1s.T, (w1s.T @ att_l1)[:, None], (w1s.T @ att_r1)[:, None]], axis=1
    ).astype(np.float16)
    w2a = np.concatenate(
        [W2.T, (W2.T @ att_l2)[:, None], (W2.T @ att_r2)[:, None]], axis=1
    ).astype(np.float32)
    b1b = np.tile(b1[None, :], (P, 1)).astype(np.float32)
    b2b = np.tile(b2[None, :], (P, 1)).astype(np.float32)
    xq = np.clip(np.round(x / xscale), -127, 127).astype(np.int8)

    in_maps = []
    for c in range(N_CORES):
        row, cls, blk, lane = per_core[c]
        key = (blk * 4 + cls) * P + lane
        order = np.argsort(key, kind="stable")
        ks = key[order]
        rs = row[order]
        cnt2 = np.bincount(ks, minlength=NBLK * 4 * P)
        starts = np.cumsum(cnt2) - cnt2
        w = np.arange(len(ks)) - starts[ks]
        bs = ks // (4 * P)
        ms = (ks // P) % 4
        ls = ks % P
        slot = (colstart[bs, ms] + w) * P + ls
        A = np.full(tot_slots16, NROWS, np.int64)  # sentinel row
        A[slot] = rs
        idx = A.reshape(-1, 16).T.astype(np.int16)  # [16, tot_slots16/16]
        xpt = np.ascontiguousarray(xq[c * SH + perms[c], :].T)
        in_maps.append(
            {
                "xpt": xpt,
                "w1a": w1a,
                "w2a": w2a,
                "b1b": b1b,
                "b2b": b2b,
                "idx": idx,
            }
        )

    meta = dict(
        N=N, SH=SH, NBLK=NBLK, IN_C=IN_C, HID=HID, OUT_C=OUT_C,
        NROWS=NROWS, Wbm=Wbm.tolist(), colstart=colstart.tolist(),
        windows=windows, totcols=totcols, perms=perms,
        idxcols=tot_slots16 // 16,
    )
    return in_maps, meta


# ------------------------------------------------------------- bass program
def _build_program(meta, num_devices=N_CORES):
    from concourse import bacc, mybir, tile
    from concourse.masks import make_identity

    f32 = mybir.dt.float32
    bf16 = mybir.dt.bfloat16
    f16 = mybir.dt.float16
    i16 = mybir.dt.int16
    i8 = mybir.dt.int8
    Alu = mybir.AluOpType
    Act = mybir.ActivationFunctionType
    AxisX = mybir.AxisListType.X

    SH = meta["SH"]
    NBLK = meta["NBLK"]
    IN_C = meta["IN_C"]
    HID = meta["HID"]
    OUT_C = meta["OUT_C"]
    NROWS = meta["NROWS"]
    Wbm = meta["Wbm"]
    colstart = meta["colstart"]
    windows = meta["windows"]
    N = meta["N"]
    idxcols = meta["idxcols"]
    totcols = meta["totcols"]
    KC = IN_C // P
    assert IN_C % P == 0 and HID == P
    SHR = SH // 4  # local packed rows

    U1 = 256       # L1 unit: f16 elems (512B): [xl*128 | a_l f32 | pad]
    U2 = 128       # L2 unit: f16 elems (256B): [h2*40 | a_l2 f32 | pad]
    AL1_F32COL = 64   # f32-view col of a_l within L1 unit
    AL2_F32COL = 20   # f32-view col of a_l2 within L2 unit

    nbs = [min(P, SH - b * P) for b in range(NBLK)]
    maxW = max(1, max(max(r) for r in Wbm))
    max_wcols = max(w for (_, w, _, _) in windows) if windows else 1

    nc = bacc.Bacc(
        "TRN2", target_bir_lowering=False, debug=False, num_devices=num_devices
    )

    xpt = nc.dram_tensor("xpt", [IN_C, SH], i8, kind="ExternalInput")
    w1a = nc.dram_tensor("w1a", [IN_C, HID + 2], f16, kind="ExternalInput")
    w2a = nc.dram_tensor("w2a", [HID, OUT_C + 2], f32, kind="ExternalInput")
    b1b = nc.dram_tensor("b1b", [P, HID], f32, kind="ExternalInput")
    b2b = nc.dram_tensor("b2b", [P, OUT_C], f32, kind="ExternalInput")
    idx = nc.dram_tensor("idx", [16, idxcols], i16, kind="ExternalInput")
    out = nc.dram_tensor("out", [SH, OUT_C], i16, kind="ExternalOutput")

    groups = [list(range(num_devices))]

    with tile.TileContext(nc) as tc:
        with (
            tc.tile_pool(name="dram", bufs=1, space="DRAM") as dpool,
            tc.tile_pool(name="const", bufs=1) as cpool,
            tc.tile_pool(name="psumT", bufs=2, space="PSUM") as psumT,
            tc.tile_pool(name="psum2", bufs=2, space="PSUM") as psum2,
        ):
            xloc = dpool.tile([SHR, 4 * U1], f16)
            xltab = dpool.tile([NROWS + 1, 4 * U1], f16)
            h2loc = dpool.tile([SHR, 4 * U2], f16)
            h2tab = dpool.tile([NROWS + 1, 4 * U2], f16)
            idxrep = dpool.tile([P, idxcols], i16)
            for k in range(8):
                nc.sync.dma_start(
                    out=idxrep[:][k * 16 : (k + 1) * 16, :], in_=idx[:, :]
                )

            ident = cpool.tile([P, P], f32)
            make_identity(nc, ident[:])
            w1a_sb = []
            for k in range(KC):
                t = cpool.tile([P, HID + 2], f16, tag=f"w1a{k}")
                nc.sync.dma_start(out=t[:], in_=w1a[k * P : (k + 1) * P, :])
                w1a_sb.append(t)
            w2a_sb = cpool.tile([P, OUT_C + 2], f32)
            nc.sync.dma_start(out=w2a_sb[:], in_=w2a[:, :])
            b1b_sb = cpool.tile([P, HID], f32)
            nc.sync.dma_start(out=b1b_sb[:], in_=b1b[:, :])
            b2b_sb = cpool.tile([P, OUT_C], f32)
            nc.sync.dma_start(out=b2b_sb[:], in_=b2b[:, :])
            ar1_sb = cpool.tile([P, NBLK], f32)
            nc.vector.memset(ar1_sb[:], 0.0)
            ar2_sb = cpool.tile([P, NBLK], f32)
            nc.vector.memset(ar2_sb[:], 0.0)
            nbias4 = cpool.tile([P, 1], f32)
            nc.vector.memset(nbias4[:], -4.0)

            # sentinel rows (all 4 units): payload=0, a_l=-1000
            s1 = cpool.tile([1, 4 * U1], f16)
            nc.vector.memset(s1[:], 0.0)
            s1f = s1[:].bitcast(f32)
            for m in range(4):
                c0 = m * (U1 // 2) + AL1_F32COL
                nc.vector.memset(s1f[:, c0 : c0 + 1], SENT_AL)
            nc.sync.dma_start(out=xltab[:][NROWS : NROWS + 1, :], in_=s1[:])
            s2 = cpool.tile([1, 4 * U2], f16)
            nc.vector.memset(s2[:], 0.0)
            s2f = s2[:].bitcast(f32)
            for m in range(4):
                c0 = m * (U2 // 2) + AL2_F32COL
                nc.vector.memset(s2f[:, c0 : c0 + 1], SENT_AL)
            nc.sync.dma_start(out=h2tab[:][NROWS : NROWS + 1, :], in_=s2[:])

            # ---------------- P1
            with (
                tc.tile_pool(name="xk", bufs=1) as xkpool,
                tc.tile_pool(name="p1", bufs=3) as p1pool,
                tc.tile_pool(name="psum1", bufs=3, space="PSUM") as psum1,
            ):
                xk = []
                for k in range(KC):
                    t8 = xkpool.tile([P, SH], i8, tag=f"xk8{k}")
                    nc.sync.dma_start(out=t8[:], in_=xpt[k * P : (k + 1) * P, :])
                    t = xkpool.tile([P, SH], f16, tag=f"xk{k}")
                    nc.vector.tensor_copy(t[:], t8[:])
                    xk.append(t)
                xlocflat = xloc[:].rearrange("a b -> (a b)")
                for t in range(NBLK):
                    nb = nbs[t]
                    ps = psum1.tile([P, HID + 2], f32, tag="ps1")
                    for k in range(KC):
                        nc.tensor.matmul(
                            ps[:nb, :],
                            lhsT=xk[k][:, t * P : t * P + nb],
                            rhs=w1a_sb[k][:],
                            start=(k == 0),
                            stop=(k == KC - 1),
                        )
                    unit = p1pool.tile([P, U1], f16, tag="unit")
                    nc.vector.memset(unit[:, HID + 2 : U1], 0.0)
                    nc.vector.tensor_copy(unit[:nb, 0:HID], ps[:nb, 0:HID])
                    uf = unit[:].bitcast(f32)
                    nc.vector.tensor_copy(
                        uf[:nb, AL1_F32COL : AL1_F32COL + 1],
                        ps[:nb, HID : HID + 1],
                    )
                    nc.vector.tensor_copy(
                        ar1_sb[:nb, t : t + 1], ps[:nb, HID + 1 : HID + 2]
                    )
                    # contiguous packed write: local node n -> bf16 elems n*U1
                    dst = xlocflat[t * P * U1 : (t * P + nb) * U1]
                    nc.sync.dma_start(
                        out=dst.rearrange("(a b) -> a b", b=U1), in_=unit[:nb, :]
                    )

            nc.gpsimd.collective_compute(
                "AllGather",
                Alu.bypass,
                replica_groups=groups,
                ins=[xloc[:].opt()],
                outs=[xltab[:][0:NROWS, :].opt()],
            )

            # ---------------- edge phase (shared between layers)
            def edge_phase(tab, UNIT, CF, alcol_f32, ar_sb, bias_sb, tab_f32,
                           finalize):
                gdt = f32 if tab_f32 else f16
                FU = UNIT if tab_f32 else UNIT // 2  # f32-view width
                with (
                    tc.tile_pool(name="gat", bufs=2) as gpool,
                    tc.tile_pool(name="acc", bufs=2) as apool,
                    tc.tile_pool(name="eb", bufs=3) as spool,
                    tc.tile_pool(name="fl", bufs=2) as flpool,
                    tc.tile_pool(name="scl", bufs=2) as sclpool,
                    tc.tile_pool(name="idxp", bufs=2) as ipool,
                    tc.tile_pool(name="arm", bufs=1) as armpool,
                ):
                    # per-column dst bias map (alpha_r of the column's block)
                    armap = armpool.tile([P, totcols], f32)
                    for b in range(NBLK):
                        for mm in range(4):
                            W = Wbm[b][mm]
                            if W == 0:
                                continue
                            s = colstart[b][mm]
                            nc.vector.tensor_copy(
                                armap[:, s : s + W],
                                ar_sb[:, b : b + 1].broadcast_to([P, W]),
                            )
                    # per-(class, block) accumulator bank: one reduce per
                    # segment, cross-class combine + division batched per
                    # group. Double-buffered so the flush of group g doesn't
                    # stall group g+1's windows.
                    def flush_group(g, acc3, accD3):
                        blist = list(range(g * GB, min((g + 1) * GB, NBLK)))
                        ng = len(blist)
                        dsum = flpool.tile([P, GB], f32, tag="dsum")
                        nc.vector.tensor_reduce(
                            out=dsum[:, 0:ng],
                            in_=accD3[:, :, 0:ng].transpose([0, 2, 1]),
                            axis=AxisX, op=Alu.add,
                        )
                        nc.vector.tensor_scalar_max(
                            dsum[:, 0:ng], dsum[:, 0:ng], 1e-16
                        )
                        rden = flpool.tile([P, GB], f32, tag="rden")
                        nc.vector.reciprocal(rden[:, 0:ng], dsum[:, 0:ng])
                        for b in blist:
                            bb = b % GB
                            aT = flpool.tile([P, CF], f32, tag="aTs")
                            nc.vector.tensor_reduce(
                                out=aT[:],
                                in_=acc3[:, :, bb, :].transpose([0, 2, 1]),
                                axis=AxisX, op=Alu.add,
                            )
                            res = flpool.tile([P, CF], f32, tag="res")
                            nc.vector.scalar_tensor_tensor(
                                out=res[:], in0=aT[:],
                                scalar=rden[:, bb : bb + 1],
                                in1=bias_sb[:], op0=Alu.mult, op1=Alu.add,
                            )
                            finalize(b, res)

                    cur_g = None
                    accT = accD = acc3 = accD3 = None
                    for (c0, wc, m, g) in windows:
                        if g != cur_g:
                            if cur_g is not None:
                                flush_group(cur_g, acc3, accD3)
                            accT = apool.tile(
                                [P, 4 * GB * CF], f16, tag="accT"
                            )
                            accD = apool.tile([P, 4 * GB], f32, tag="accD")
                            acc3 = accT[:].rearrange(
                                "p (m b c) -> p m b c", m=4, b=GB
                            )
                            accD3 = accD[:].rearrange(
                                "p (m b) -> p m b", m=4
                            )
                            nc.vector.memset(accT[:], 0.0)
                            nc.vector.memset(accD[:], 0.0)
                            cur_g = g
                        gt = gpool.tile([P, max_wcols * UNIT], gdt, tag="gt")
                        islab = ipool.tile([P, WCOLS * 8], i16, tag="islab")
                        nc.sync.dma_start(
                            out=islab[:, 0 : wc * 8],
                            in_=idxrep[:][:, c0 * 8 : (c0 + wc) * 8],
                        )
                        nidx = wc * P
                        nc.gpsimd.dma_gather(
                            out_ap=gt[:, 0 : wc * UNIT].rearrange(
                                "p (w c) -> p w c", c=UNIT
                            ),
                            in_ap=tab[:][:, m * UNIT : (m + 1) * UNIT],
                            idxs_ap=islab[:, 0 : wc * 8],
                            num_idxs=nidx,
                            num_idxs_reg=nidx,
                            elem_size=UNIT,
                            elem_step=4 * UNIT,
                            single_packet=False,
                        )
                        g3 = gt[:, 0 : wc * UNIT].rearrange(
                            "p (w c) -> p w c", c=UNIT
                        )
                        if tab_f32:
                            g3f = g3
                        else:
                            g3f = gt[:, 0 : wc * UNIT].bitcast(f32).rearrange(
                                "p (w c) -> p w c", c=FU
                            )
                        # fused whole-window: z = a_l + a_r; leaky; exp; msgs
                        alv = g3f[:, :, alcol_f32 : alcol_f32 + 1].squeeze(2)
                        zt = spool.tile([P, WCOLS], f32, tag="z")
                        z = zt[:, 0:wc]
                        nc.vector.tensor_tensor(
                            out=z, in0=alv, in1=armap[:, c0 : c0 + wc],
                            op=Alu.add,
                        )
                        et = spool.tile([P, WCOLS], f32, tag="e")
                        e = et[:, 0:wc]
                        nc.vector.scalar_tensor_tensor(
                            out=e, in0=z, scalar=0.2, in1=z,
                            op0=Alu.mult, op1=Alu.max,
                        )
                        ext = spool.tile([P, WCOLS], gdt, tag="ex")
                        ex = ext[:, 0:wc]
                        nc.scalar.activation(ex, e, Act.Exp, bias=nbias4[:])
                        scl = sclpool.tile([P, max_wcols * CF], gdt, tag="scl")
                        scl3 = scl[:, 0 : wc * CF].rearrange(
                            "p (w c) -> p w c", c=CF
                        )
                        nc.vector.tensor_tensor(
                            out=scl3,
                            in0=g3[:, :, 0:CF],
                            in1=ex.unsqueeze(2).broadcast_to([P, wc, CF]),
                            op=Alu.mult,
                        )
                        for b in range(NBLK):
                            W = Wbm[b][m]
                            s = colstart[b][m]
                            if W == 0 or s < c0 or s >= c0 + wc:
                                continue
                            o = s - c0
                            slot = m * GB + (b % GB)
                            with nc.allow_low_precision(
                                reason="short f16 segment sums; normalized "
                                "by f32 denominators downstream"
                            ):
                                nc.vector.tensor_reduce(
                                    out=accT[:, slot * CF : (slot + 1) * CF],
                                    in_=scl3[:, o : o + W, :].transpose(
                                        [0, 2, 1]
                                    ),
                                    axis=AxisX, op=Alu.add,
                                )
                            nc.vector.tensor_reduce(
                                out=accD[:, slot : slot + 1],
                                in_=ex[:, o : o + W],
                                axis=AxisX, op=Alu.add,
                            )
                    if cur_g is not None:
                        flush_group(cur_g, acc3, accD3)

            # ---------------- L1 finalize: ELU + fused W2 projection
            with tc.tile_pool(name="fin1", bufs=3) as fpool:
                h2locflat = h2loc[:].rearrange("a b -> (a b)")

                def fin1(b, hpre):
                    nb = nbs[b]
                    xm = fpool.tile([P, HID], f32, tag="xm")
                    nc.vector.tensor_scalar_min(xm[:], hpre[:], 0.0)
                    em = fpool.tile([P, HID], f32, tag="em")
                    nc.scalar.activation(em[:], xm[:], Act.Exp)
                    h = fpool.tile([P, HID], f32, tag="h")
                    nc.vector.scalar_tensor_tensor(
                        out=h[:], in0=hpre[:], scalar=0.0, op0=Alu.max,
                        in1=em[:], op1=Alu.add,
                    )
                    nc.vector.tensor_scalar_add(h[:], h[:], -1.0)
                    hT_ps = psumT.tile([P, P], f32, tag="hT")
                    nc.tensor.transpose(hT_ps[:], h[:], ident[:])
                    hT = fpool.tile([P, P], f32, tag="hTs")
                    nc.vector.tensor_copy(hT[:], hT_ps[:])
                    h2ps = psum2.tile([P, OUT_C + 2], f32, tag="h2ps")
                    nc.tensor.matmul(
                        h2ps[:nb, :], lhsT=hT[:, :nb], rhs=w2a_sb[:],
                        start=True, stop=True,
                    )
                    unit = fpool.tile([P, U2], f16, tag="u2")
                    nc.vector.memset(unit[:, OUT_C + 2 : U2], 0.0)
                    nc.vector.tensor_copy(
                        unit[:nb, 0:OUT_C], h2ps[:nb, 0:OUT_C]
                    )
                    uf2 = unit[:].bitcast(f32)
                    nc.vector.tensor_copy(
                        uf2[:nb, AL2_F32COL : AL2_F32COL + 1],
                        h2ps[:nb, OUT_C : OUT_C + 1],
                    )
                    nc.vector.tensor_copy(
                        ar2_sb[:nb, b : b + 1], h2ps[:nb, OUT_C + 1 : OUT_C + 2]
                    )
                    dstf = h2locflat[b * P * U2 : (b * P + nb) * U2]
                    nc.sync.dma_start(
                        out=dstf.rearrange("(a b) -> a b", b=U2),
                        in_=unit[:nb, :],
                    )

                edge_phase(
                    xltab, U1, HID, AL1_F32COL, ar1_sb, b1b_sb, False, fin1
                )

            nc.gpsimd.collective_compute(
                "AllGather",
                Alu.bypass,
                replica_groups=groups,
                ins=[h2loc[:].opt()],
                outs=[h2tab[:][0:NROWS, :].opt()],
            )

            # ---------------- L2 finalize: log_softmax + output
            with tc.tile_pool(name="fin2", bufs=3) as f2pool:

                def fin2(b, logits):
                    nb = nbs[b]
                    nm = f2pool.tile([P, 1], f32, tag="nm")
                    nc.vector.tensor_reduce(
                        out=nm[:], in_=logits[:], axis=AxisX, op=Alu.max,
                        negate=True,
                    )
                    exl = f2pool.tile([P, OUT_C], f32, tag="exl")
                    ssum = f2pool.tile([P, 1], f32, tag="ssum")
                    nc.scalar.activation(
                        exl[:], logits[:], Act.Exp, bias=nm[:],
                        accum_out=ssum[:],
                    )
                    lns = f2pool.tile([P, 1], f32, tag="lns")
                    nc.scalar.activation(lns[:], ssum[:], Act.Ln)
                    cc = f2pool.tile([P, 1], f32, tag="cc")
                    nc.vector.tensor_tensor(
                        out=cc[:], in0=nm[:], in1=lns[:], op=Alu.subtract
                    )
                    fin = f2pool.tile([P, OUT_C], i16, tag="fin")
                    nc.vector.tensor_scalar(
                        out=fin[:], in0=logits[:], scalar1=cc[:],
                        scalar2=S_OUT, op0=Alu.add, op1=Alu.mult,
                    )
                    nc.sync.dma_start(
                        out=out[b * P : b * P + nb, :], in_=fin[:nb, :]
                    )

                edge_phase(h2tab, U2, OUT_C, AL2_F32COL, ar2_sb, b2b_sb, False, fin2)

    nc.compile()
    return nc


# ------------------------------------------------------------------- driver
def kernel(x, edge_index, W1, att_l1, att_r1, b1, W2, att_l2, att_r2, b2):
    from concourse.bass_utils import run_bass_kernel_spmd

    in_maps, meta = _host_prep(
        x, edge_index, W1, att_l1, att_r1, b1, W2, att_l2, att_r2, b2
    )
    nc = _build_program(meta)
    res = run_bass_kernel_spmd(nc, in_maps, core_ids=list(range(N_CORES)))
    N, SH = meta["N"], meta["SH"]
    OUT_C = meta["OUT_C"]
    full = np.empty((N, OUT_C), np.float32)
    for c in range(N_CORES):
        full[c * SH + meta["perms"][c]] = (
            res.results[c]["out"].astype(np.float32) / S_OUT
        )
    return full



# revision 28
# speedup vs baseline: 1.0744x; 1.0744x over previous
"""Two-layer GAT on 8 Trainium2 NeuronCores.

Strategy (dst-partitioned edge parallelism, degree-sorted blocks):
  - Core c owns nodes [c*SH, (c+1)*SH) for the feature matmul and as edge
    destinations, so the segment softmax over incoming edges is core-local.
  - Per core, dst nodes are in-degree sorted into blocks of 128 (one node
    per SBUF partition); a node's incoming edges lie along the free dim.
  - x ships int8 (global scale folded into W1); the P1 matmul runs f16.
  - Edge gathers use nc.gpsimd.dma_gather (int16 indices). The gather
    table packs 4 nodes per row (row = gpos//4, class = gpos%4) so row ids
    fit in int16; each class is a strided column slice of the table.
    Edge slots are grouped per (block, class-of-src) segment, padded to
    the cross-core max; pad slots gather a sentinel unit whose
    alpha_l = -1000 so exp() -> 0. A host-side greedy balances each
    node's class (pos%4) against its out-edges' destinations, shrinking
    the per-(block,class) max widths ~27%.
  - Layer-1 units are [xl f16 x128 | alpha_l f32 | pad] (512B); layer-2
    units are [h2 f16 x40 | alpha_l2 f32 | pad] (256B).
  - Per gather window the whole slab is processed with fused ops:
    z = alpha_l + armap (per-column alpha_r map), leaky, exp(z-4)
    (shift is softmax-invariant, keeps f16 partials small), payload*ex.
    Each (block,class) segment does one f16 reduce into a per-class
    accumulator bank; cross-class combine + reciprocal are batched per
    group of GB blocks at flush time.
  - The layer-2 projection (W2, att vectors) is fused into the layer-1
    block epilogue (PE transpose + matmul); an 8-core AllGather exchanges
    the packed tables between layers. Output is int16 (x512 scale).
"""

import sys

for _p in ("/opt/trn_rl_repo",):
    if _p not in sys.path:
        sys.path.insert(0, _p)

import numpy as np

try:
    import tempfile

    import jax

    _cc_dir = tempfile.gettempdir() + "/jax_cc_cache"
    jax.config.update("jax_compilation_cache_dir", _cc_dir)
    jax.config.update("jax_persistent_cache_min_entry_size_bytes", -1)
    jax.config.update("jax_persistent_cache_min_compile_time_secs", 0)
except Exception:
    pass

N_CORES = 8
P = 128
GB = 33        # blocks per sweep group
WCOLS = 48     # max gather-window width in slot-columns (128 edges each)
SENT_AL = -1000.0
S_OUT = 512.0  # int16 output scale


# ---------------------------------------------------------------- host prep
def _host_prep(x, edge_index, W1, att_l1, att_r1, b1, W2, att_l2, att_r2, b2):
    x = np.asarray(x, np.float32)
    ei = np.asarray(edge_index).astype(np.int64)
    W1 = np.asarray(W1, np.float32)
    W2 = np.asarray(W2, np.float32)
    att_l1 = np.asarray(att_l1, np.float32)
    att_r1 = np.asarray(att_r1, np.float32)
    att_l2 = np.asarray(att_l2, np.float32)
    att_r2 = np.asarray(att_r2, np.float32)
    b1 = np.asarray(b1, np.float32)
    b2 = np.asarray(b2, np.float32)

    N, IN_C = x.shape
    HID = W1.shape[0]
    OUT_C = W2.shape[0]
    assert N % (N_CORES * 4) == 0
    SH = N // N_CORES
    NBLK = -(-SH // P)
    NROWS = N // 4  # packed table rows
    src, dst = ei[0], ei[1]
    owner = dst // SH

    perms = []
    for c in range(N_CORES):
        m = owner == c
        d0 = dst[m] - c * SH
        deg = np.bincount(d0, minlength=SH)
        perms.append(np.argsort(deg, kind="stable"))

    # --- class balancing: greedily pick each node's table class (pos%4)
    # to flatten per-(dst,class) incoming-edge counts, shrinking the
    # per-(block,class) max widths (= gather padding). Blocks keep their
    # degree-sorted membership; only the within-block lane order changes.
    pos = np.empty(N, np.int64)
    for c in range(N_CORES):
        pos[c * SH + perms[c]] = np.arange(SH)
    outdeg = np.bincount(src, minlength=N)
    indeg = np.bincount(dst, minlength=N)
    fair = np.ceil(indeg / 4.0).astype(np.int32)
    cnt4 = np.zeros((N, 4), np.int32)
    order_e = np.argsort(src, kind="stable")
    ed = dst[order_e]
    starts = np.searchsorted(src[order_e], np.arange(N))
    ends = np.searchsorted(src[order_e], np.arange(N) + 1)
    blk_key = (np.arange(N) // SH) * NBLK + (pos % SH) // P
    bsz = np.bincount(blk_key, minlength=N_CORES * NBLK)
    cap = np.repeat(bsz // 4, 4).reshape(-1, 4).astype(np.int64).copy()
    cls = np.empty(N, np.int64)
    for u in np.argsort(-outdeg, kind="stable"):
        e0, e1 = starts[u], ends[u]
        bk = blk_key[u]
        if e1 > e0:
            vs = ed[e0:e1]
            c4 = cnt4[vs]
            over = np.maximum(c4 + 1 - fair[vs][:, None], 0).sum(axis=0)
            cost = over * 1000.0 + c4.sum(axis=0)
        else:
            cost = np.zeros(4)
        cost = np.where(cap[bk] > 0, cost, np.inf)
        mm = int(np.argmin(cost))
        cls[u] = mm
        cap[bk, mm] -= 1
        if e1 > e0:
            cnt4[vs, mm] += 1
    # rebuild perms: within each block, lane l holds a class l%4 node
    invperms = []
    for c in range(N_CORES):
        newp = np.empty(SH, np.int64)
        cl_loc = cls[c * SH + perms[c]]  # class per old position
        for b in range(NBLK):
            lo = b * P
            hi = min(lo + P, SH)
            blknodes = perms[c][lo:hi]
            blkcls = cl_loc[lo:hi]
            nb = hi - lo
            for mm in range(4):
                sel = blknodes[blkcls == mm]
                newp[lo + mm : lo + nb : 4] = sel
        perms[c] = newp
        inv = np.empty(SH, np.int64)
        inv[newp] = np.arange(SH)
        invperms.append(inv)

    gpos = np.empty(N, np.int64)
    for c in range(N_CORES):
        gpos[c * SH + perms[c]] = c * SH + np.arange(SH)

    # per (block, class) widths, common max across cores
    Wbm = np.zeros((NBLK, 4), np.int64)
    per_core = []
    for c in range(N_CORES):
        m = owner == c
        s_c = src[m]
        d0 = dst[m] - c * SH
        pos = invperms[c][d0]         # dst slot position (block*128+lane)
        g = gpos[s_c]                 # src table position
        cls = (g % 4).astype(np.int64)
        row = g // 4
        blk = pos // P
        lane = pos % P
        cnt = np.zeros((NBLK, 4, P), np.int64)
        np.add.at(cnt, (blk, cls, lane), 1)
        Wbm = np.maximum(Wbm, cnt.max(axis=2))
        per_core.append((row, cls, blk, lane))

    # grid: groups of GB blocks, class-major inside the group
    colstart = np.zeros((NBLK, 4), np.int64)
    windows = []  # (colstart_global, ncols, class, group) per gather call
    col = 0
    b0 = 0
    grp = 0
    while b0 < NBLK:
        b1_ = min(b0 + GB, NBLK)
        for m in range(4):
            wstart = col
            wcols = 0
            for b in range(b0, b1_):
                w = int(Wbm[b, m])
                if wcols + w > WCOLS and wcols > 0:
                    windows.append((wstart, wcols, m, grp))
                    wstart = col
                    wcols = 0
                colstart[b, m] = col
                col += w
                wcols += w
            if wcols > 0:
                windows.append((wstart, wcols, m, grp))
        b0 = b1_
        grp += 1
    totcols = int(col)
    tot_slots = totcols * P
    tot_slots16 = -(-tot_slots // 16) * 16

    # int8-quantize x with a global scale folded into W1 (xl = x_q @ (s*W1).T)
    xscale = max(float(np.abs(x).max()), 1e-8) / 127.0
    w1s = (W1 * xscale).astype(np.float32)
    w1a = np.concatenate(
        [w1s.T, (w1s.T @ att_l1)[:, None], (w1s.T @ att_r1)[:, None]], axis=1
    ).astype(np.float16)
    w2a = np.concatenate(
        [W2.T, (W2.T @ att_l2)[:, None], (W2.T @ att_r2)[:, None]], axis=1
    ).astype(np.float32)
    b1b = np.tile(b1[None, :], (P, 1)).astype(np.float32)
    b2b = np.tile(b2[None, :], (P, 1)).astype(np.float32)
    xq = np.clip(np.round(x / xscale), -127, 127).astype(np.int8)

    in_maps = []
    for c in range(N_CORES):
        row, cls, blk, lane = per_core[c]
        key = (blk * 4 + cls) * P + lane
        order = np.argsort(key, kind="stable")
        ks = key[order]
        rs = row[order]
        cnt2 = np.bincount(ks, minlength=NBLK * 4 * P)
        starts = np.cumsum(cnt2) - cnt2
        w = np.arange(len(ks)) - starts[ks]
        bs = ks // (4 * P)
        ms = (ks // P) % 4
        ls = ks % P
        slot = (colstart[bs, ms] + w) * P + ls
        A = np.full(tot_slots16, NROWS, np.int64)  # sentinel row
        A[slot] = rs
        idx = A.reshape(-1, 16).T.astype(np.int16)  # [16, tot_slots16/16]
        xpt = np.ascontiguousarray(xq[c * SH + perms[c], :].T)
        in_maps.append(
            {
                "xpt": xpt,
                "w1a": w1a,
                "w2a": w2a,
                "b1b": b1b,
                "b2b": b2b,
                "idx": idx,
            }
        )

    meta = dict(
        N=N, SH=SH, NBLK=NBLK, IN_C=IN_C, HID=HID, OUT_C=OUT_C,
        NROWS=NROWS, Wbm=Wbm.tolist(), colstart=colstart.tolist(),
        windows=windows, totcols=totcols, perms=perms,
        idxcols=tot_slots16 // 16,
    )
    return in_maps, meta


# ------------------------------------------------------------- bass program
def _build_program(meta, num_devices=N_CORES):
    from concourse import bacc, mybir, tile
    from concourse.masks import make_identity

    f32 = mybir.dt.float32
    bf16 = mybir.dt.bfloat16
    f16 = mybir.dt.float16
    i16 = mybir.dt.int16
    i8 = mybir.dt.int8
    Alu = mybir.AluOpType
    Act = mybir.ActivationFunctionType
    AxisX = mybir.AxisListType.X

    SH = meta["SH"]
    NBLK = meta["NBLK"]
    IN_C = meta["IN_C"]
    HID = meta["HID"]
    OUT_C = meta["OUT_C"]
    NROWS = meta["NROWS"]
    Wbm = meta["Wbm"]
    colstart = meta["colstart"]
    windows = meta["windows"]
    N = meta["N"]
    idxcols = meta["idxcols"]
    totcols = meta["totcols"]
    KC = IN_C // P
    assert IN_C % P == 0 and HID == P
    SHR = SH // 4  # local packed rows

    U1 = 256       # L1 unit: f16 elems (512B): [xl*128 | a_l f32 | pad]
    U2 = 128       # L2 unit: f16 elems (256B): [h2*40 | a_l2 f32 | pad]
    AL1_F32COL = 64   # f32-view col of a_l within L1 unit
    AL2_F32COL = 20   # f32-view col of a_l2 within L2 unit

    nbs = [min(P, SH - b * P) for b in range(NBLK)]
    maxW = max(1, max(max(r) for r in Wbm))
    max_wcols = max(w for (_, w, _, _) in windows) if windows else 1

    nc = bacc.Bacc(
        "TRN2", target_bir_lowering=False, debug=False, num_devices=num_devices
    )

    xpt = nc.dram_tensor("xpt", [IN_C, SH], i8, kind="ExternalInput")
    w1a = nc.dram_tensor("w1a", [IN_C, HID + 2], f16, kind="ExternalInput")
    w2a = nc.dram_tensor("w2a", [HID, OUT_C + 2], f32, kind="ExternalInput")
    b1b = nc.dram_tensor("b1b", [P, HID], f32, kind="ExternalInput")
    b2b = nc.dram_tensor("b2b", [P, OUT_C], f32, kind="ExternalInput")
    idx = nc.dram_tensor("idx", [16, idxcols], i16, kind="ExternalInput")
    out = nc.dram_tensor("out", [SH, OUT_C], i16, kind="ExternalOutput")

    groups = [list(range(num_devices))]

    with tile.TileContext(nc) as tc:
        with (
            tc.tile_pool(name="dram", bufs=1, space="DRAM") as dpool,
            tc.tile_pool(name="const", bufs=1) as cpool,
            tc.tile_pool(name="psumT", bufs=2, space="PSUM") as psumT,
            tc.tile_pool(name="psum2", bufs=2, space="PSUM") as psum2,
        ):
            xloc = dpool.tile([SHR, 4 * U1], f16)
            xltab = dpool.tile([NROWS + 1, 4 * U1], f16)
            h2loc = dpool.tile([SHR, 4 * U2], f16)
            h2tab = dpool.tile([NROWS + 1, 4 * U2], f16)
            idxrep = dpool.tile([P, idxcols], i16)
            for k in range(8):
                nc.sync.dma_start(
                    out=idxrep[:][k * 16 : (k + 1) * 16, :], in_=idx[:, :]
                )

            ident = cpool.tile([P, P], f32)
            make_identity(nc, ident[:])
            w1a_sb = []
            for k in range(KC):
                t = cpool.tile([P, HID + 2], f16, tag=f"w1a{k}")
                nc.sync.dma_start(out=t[:], in_=w1a[k * P : (k + 1) * P, :])
                w1a_sb.append(t)
            w2a_sb = cpool.tile([P, OUT_C + 2], f32)
            nc.sync.dma_start(out=w2a_sb[:], in_=w2a[:, :])
            b1b_sb = cpool.tile([P, HID], f32)
            nc.sync.dma_start(out=b1b_sb[:], in_=b1b[:, :])
            b2b_sb = cpool.tile([P, OUT_C], f32)
            nc.sync.dma_start(out=b2b_sb[:], in_=b2b[:, :])
            ar1_sb = cpool.tile([P, NBLK], f32)
            nc.vector.memset(ar1_sb[:], 0.0)
            ar2_sb = cpool.tile([P, NBLK], f32)
            nc.vector.memset(ar2_sb[:], 0.0)
            nbias4 = cpool.tile([P, 1], f32)
            nc.vector.memset(nbias4[:], -4.0)

            # sentinel rows (all 4 units): payload=0, a_l=-1000
            s1 = cpool.tile([1, 4 * U1], f16)
            nc.vector.memset(s1[:], 0.0)
            s1f = s1[:].bitcast(f32)
            for m in range(4):
                c0 = m * (U1 // 2) + AL1_F32COL
                nc.vector.memset(s1f[:, c0 : c0 + 1], SENT_AL)
            nc.sync.dma_start(out=xltab[:][NROWS : NROWS + 1, :], in_=s1[:])
            s2 = cpool.tile([1, 4 * U2], f16)
            nc.vector.memset(s2[:], 0.0)
            s2f = s2[:].bitcast(f32)
            for m in range(4):
                c0 = m * (U2 // 2) + AL2_F32COL
                nc.vector.memset(s2f[:, c0 : c0 + 1], SENT_AL)
            nc.sync.dma_start(out=h2tab[:][NROWS : NROWS + 1, :], in_=s2[:])

            # ---------------- P1
            with (
                tc.tile_pool(name="xk", bufs=1) as xkpool,
                tc.tile_pool(name="p1", bufs=3) as p1pool,
                tc.tile_pool(name="psum1", bufs=3, space="PSUM") as psum1,
            ):
                xk = []
                for k in range(KC):
                    t8 = xkpool.tile([P, SH], i8, tag=f"xk8{k}")
                    nc.sync.dma_start(out=t8[:], in_=xpt[k * P : (k + 1) * P, :])
                    t = xkpool.tile([P, SH], f16, tag=f"xk{k}")
                    nc.vector.tensor_copy(t[:], t8[:])
                    xk.append(t)
                xlocflat = xloc[:].rearrange("a b -> (a b)")
                for t in range(NBLK):
                    nb = nbs[t]
                    ps = psum1.tile([P, HID + 2], f32, tag="ps1")
                    for k in range(KC):
                        nc.tensor.matmul(
                            ps[:nb, :],
                            lhsT=xk[k][:, t * P : t * P + nb],
                            rhs=w1a_sb[k][:],
                            start=(k == 0),
                            stop=(k == KC - 1),
                        )
                    unit = p1pool.tile([P, U1], f16, tag="unit")
                    nc.vector.memset(unit[:, HID + 2 : U1], 0.0)
                    nc.vector.tensor_copy(unit[:nb, 0:HID], ps[:nb, 0:HID])
                    uf = unit[:].bitcast(f32)
                    nc.vector.tensor_copy(
                        uf[:nb, AL1_F32COL : AL1_F32COL + 1],
                        ps[:nb, HID : HID + 1],
                    )
                    nc.vector.tensor_copy(
                        ar1_sb[:nb, t : t + 1], ps[:nb, HID + 1 : HID + 2]
                    )
                    # contiguous packed write: local node n -> bf16 elems n*U1
                    dst = xlocflat[t * P * U1 : (t * P + nb) * U1]
                    nc.sync.dma_start(
                        out=dst.rearrange("(a b) -> a b", b=U1), in_=unit[:nb, :]
                    )

            nc.gpsimd.collective_compute(
                "AllGather",
                Alu.bypass,
                replica_groups=groups,
                ins=[xloc[:].opt()],
                outs=[xltab[:][0:NROWS, :].opt()],
            )

            # ---------------- edge phase (shared between layers)
            def edge_phase(tab, UNIT, CF, alcol_f32, ar_sb, bias_sb, tab_f32,
                           finalize):
                gdt = f32 if tab_f32 else f16
                FU = UNIT if tab_f32 else UNIT // 2  # f32-view width
                with (
                    tc.tile_pool(name="gat", bufs=4) as gpool,
                    tc.tile_pool(name="acc", bufs=1) as apool,
                    tc.tile_pool(name="eb", bufs=3) as spool,
                    tc.tile_pool(name="fl", bufs=2) as flpool,
                    tc.tile_pool(name="scl", bufs=2) as sclpool,
                    tc.tile_pool(name="idxp", bufs=4) as ipool,
                    tc.tile_pool(name="arm", bufs=1) as armpool,
                ):
                    # per-column dst bias map (alpha_r of the column's block)
                    armap = armpool.tile([P, totcols], f32)
                    for b in range(NBLK):
                        for mm in range(4):
                            W = Wbm[b][mm]
                            if W == 0:
                                continue
                            s = colstart[b][mm]
                            nc.vector.tensor_copy(
                                armap[:, s : s + W],
                                ar_sb[:, b : b + 1].broadcast_to([P, W]),
                            )
                    # per-(class, block) accumulator bank: one reduce per
                    # segment, cross-class combine + division batched per
                    # group. Double-buffered so the flush of group g doesn't
                    # stall group g+1's windows.
                    def flush_group(g, acc3, accD3):
                        blist = list(range(g * GB, min((g + 1) * GB, NBLK)))
                        ng = len(blist)
                        dsum = flpool.tile([P, GB], f32, tag="dsum")
                        nc.vector.tensor_reduce(
                            out=dsum[:, 0:ng],
                            in_=accD3[:, :, 0:ng].transpose([0, 2, 1]),
                            axis=AxisX, op=Alu.add,
                        )
                        nc.vector.tensor_scalar_max(
                            dsum[:, 0:ng], dsum[:, 0:ng], 1e-16
                        )
                        rden = flpool.tile([P, GB], f32, tag="rden")
                        nc.vector.reciprocal(rden[:, 0:ng], dsum[:, 0:ng])
                        for b in blist:
                            bb = b % GB
                            aT = flpool.tile([P, CF], f32, tag="aTs")
                            nc.vector.tensor_reduce(
                                out=aT[:],
                                in_=acc3[:, :, bb, :].transpose([0, 2, 1]),
                                axis=AxisX, op=Alu.add,
                            )
                            res = flpool.tile([P, CF], f32, tag="res")
                            nc.vector.scalar_tensor_tensor(
                                out=res[:], in0=aT[:],
                                scalar=rden[:, bb : bb + 1],
                                in1=bias_sb[:], op0=Alu.mult, op1=Alu.add,
                            )
                            finalize(b, res)

                    cur_g = None
                    accT = accD = acc3 = accD3 = None
                    for (c0, wc, m, g) in windows:
                        if g != cur_g:
                            if cur_g is not None:
                                flush_group(cur_g, acc3, accD3)
                            accT = apool.tile(
                                [P, 4 * GB * CF], f16, tag="accT"
                            )
                            accD = apool.tile([P, 4 * GB], f32, tag="accD")
                            acc3 = accT[:].rearrange(
                                "p (m b c) -> p m b c", m=4, b=GB
                            )
                            accD3 = accD[:].rearrange(
                                "p (m b) -> p m b", m=4
                            )
                            nc.vector.memset(accT[:], 0.0)
                            nc.vector.memset(accD[:], 0.0)
                            cur_g = g
                        gt = gpool.tile([P, max_wcols * UNIT], gdt, tag="gt")
                        islab = ipool.tile([P, WCOLS * 8], i16, tag="islab")
                        nc.sync.dma_start(
                            out=islab[:, 0 : wc * 8],
                            in_=idxrep[:][:, c0 * 8 : (c0 + wc) * 8],
                        )
                        nidx = wc * P
                        nc.gpsimd.dma_gather(
                            out_ap=gt[:, 0 : wc * UNIT].rearrange(
                                "p (w c) -> p w c", c=UNIT
                            ),
                            in_ap=tab[:][:, m * UNIT : (m + 1) * UNIT],
                            idxs_ap=islab[:, 0 : wc * 8],
                            num_idxs=nidx,
                            num_idxs_reg=nidx,
                            elem_size=UNIT,
                            elem_step=4 * UNIT,
                            single_packet=False,
                        )
                        g3 = gt[:, 0 : wc * UNIT].rearrange(
                            "p (w c) -> p w c", c=UNIT
                        )
                        if tab_f32:
                            g3f = g3
                        else:
                            g3f = gt[:, 0 : wc * UNIT].bitcast(f32).rearrange(
                                "p (w c) -> p w c", c=FU
                            )
                        # fused whole-window: z = a_l + a_r; leaky; exp; msgs
                        alv = g3f[:, :, alcol_f32 : alcol_f32 + 1].squeeze(2)
                        zt = spool.tile([P, WCOLS], f32, tag="z")
                        z = zt[:, 0:wc]
                        nc.vector.tensor_tensor(
                            out=z, in0=alv, in1=armap[:, c0 : c0 + wc],
                            op=Alu.add,
                        )
                        et = spool.tile([P, WCOLS], f32, tag="e")
                        e = et[:, 0:wc]
                        nc.vector.scalar_tensor_tensor(
                            out=e, in0=z, scalar=0.2, in1=z,
                            op0=Alu.mult, op1=Alu.max,
                        )
                        ext = spool.tile([P, WCOLS], gdt, tag="ex")
                        ex = ext[:, 0:wc]
                        nc.scalar.activation(ex, e, Act.Exp, bias=nbias4[:])
                        scl = sclpool.tile([P, max_wcols * CF], gdt, tag="scl")
                        scl3 = scl[:, 0 : wc * CF].rearrange(
                            "p (w c) -> p w c", c=CF
                        )
                        nc.vector.tensor_tensor(
                            out=scl3,
                            in0=g3[:, :, 0:CF],
                            in1=ex.unsqueeze(2).broadcast_to([P, wc, CF]),
                            op=Alu.mult,
                        )
                        for b in range(NBLK):
                            W = Wbm[b][m]
                            s = colstart[b][m]
                            if W == 0 or s < c0 or s >= c0 + wc:
                                continue
                            o = s - c0
                            slot = m * GB + (b % GB)
                            with nc.allow_low_precision(
                                reason="short f16 segment sums; normalized "
                                "by f32 denominators downstream"
                            ):
                                nc.vector.tensor_reduce(
                                    out=accT[:, slot * CF : (slot + 1) * CF],
                                    in_=scl3[:, o : o + W, :].transpose(
                                        [0, 2, 1]
                                    ),
                                    axis=AxisX, op=Alu.add,
                                )
                            nc.vector.tensor_reduce(
                                out=accD[:, slot : slot + 1],
                                in_=ex[:, o : o + W],
                                axis=AxisX, op=Alu.add,
                            )
                    if cur_g is not None:
                        flush_group(cur_g, acc3, accD3)

            # ---------------- L1 finalize: ELU + fused W2 projection
            with tc.tile_pool(name="fin1", bufs=3) as fpool:
                h2locflat = h2loc[:].rearrange("a b -> (a b)")

                def fin1(b, hpre):
                    nb = nbs[b]
                    xm = fpool.tile([P, HID], f32, tag="xm")
                    nc.vector.tensor_scalar_min(xm[:], hpre[:], 0.0)
                    em = fpool.tile([P, HID], f32, tag="em")
                    nc.scalar.activation(em[:], xm[:], Act.Exp)
                    h = fpool.tile([P, HID], f32, tag="h")
                    nc.vector.scalar_tensor_tensor(
                        out=h[:], in0=hpre[:], scalar=0.0, op0=Alu.max,
                        in1=em[:], op1=Alu.add,
                    )
                    nc.vector.tensor_scalar_add(h[:], h[:], -1.0)
                    hT_ps = psumT.tile([P, P], f32, tag="hT")
                    nc.tensor.transpose(hT_ps[:], h[:], ident[:])
                    hT = fpool.tile([P, P], f32, tag="hTs")
                    nc.vector.tensor_copy(hT[:], hT_ps[:])
                    h2ps = psum2.tile([P, OUT_C + 2], f32, tag="h2ps")
                    nc.tensor.matmul(
                        h2ps[:nb, :], lhsT=hT[:, :nb], rhs=w2a_sb[:],
                        start=True, stop=True,
                    )
                    unit = fpool.tile([P, U2], f16, tag="u2")
                    nc.vector.memset(unit[:, OUT_C + 2 : U2], 0.0)
                    nc.vector.tensor_copy(
                        unit[:nb, 0:OUT_C], h2ps[:nb, 0:OUT_C]
                    )
                    uf2 = unit[:].bitcast(f32)
                    nc.vector.tensor_copy(
                        uf2[:nb, AL2_F32COL : AL2_F32COL + 1],
                        h2ps[:nb, OUT_C : OUT_C + 1],
                    )
                    nc.vector.tensor_copy(
                        ar2_sb[:nb, b : b + 1], h2ps[:nb, OUT_C + 1 : OUT_C + 2]
                    )
                    dstf = h2locflat[b * P * U2 : (b * P + nb) * U2]
                    nc.sync.dma_start(
                        out=dstf.rearrange("(a b) -> a b", b=U2),
                        in_=unit[:nb, :],
                    )

                edge_phase(
                    xltab, U1, HID, AL1_F32COL, ar1_sb, b1b_sb, False, fin1
                )

            nc.gpsimd.collective_compute(
                "AllGather",
                Alu.bypass,
                replica_groups=groups,
                ins=[h2loc[:].opt()],
                outs=[h2tab[:][0:NROWS, :].opt()],
            )

            # ---------------- L2 finalize: log_softmax + output
            with tc.tile_pool(name="fin2", bufs=3) as f2pool:

                def fin2(b, logits):
                    nb = nbs[b]
                    nm = f2pool.tile([P, 1], f32, tag="nm")
                    nc.vector.tensor_reduce(
                        out=nm[:], in_=logits[:], axis=AxisX, op=Alu.max,
                        negate=True,
                    )
                    exl = f2pool.tile([P, OUT_C], f32, tag="exl")
                    ssum = f2pool.tile([P, 1], f32, tag="ssum")
                    nc.scalar.activation(
                        exl[:], logits[:], Act.Exp, bias=nm[:],
                        accum_out=ssum[:],
                    )
                    lns = f2pool.tile([P, 1], f32, tag="lns")
                    nc.scalar.activation(lns[:], ssum[:], Act.Ln)
                    cc = f2pool.tile([P, 1], f32, tag="cc")
                    nc.vector.tensor_tensor(
                        out=cc[:], in0=nm[:], in1=lns[:], op=Alu.subtract
                    )
                    fin = f2pool.tile([P, OUT_C], i16, tag="fin")
                    nc.vector.tensor_scalar(
                        out=fin[:], in0=logits[:], scalar1=cc[:],
                        scalar2=S_OUT, op0=Alu.add, op1=Alu.mult,
                    )
                    nc.sync.dma_start(
                        out=out[b * P : b * P + nb, :], in_=fin[:nb, :]
                    )

                edge_phase(h2tab, U2, OUT_C, AL2_F32COL, ar2_sb, b2b_sb, False, fin2)

    nc.compile()
    return nc


# ------------------------------------------------------------------- driver
def kernel(x, edge_index, W1, att_l1, att_r1, b1, W2, att_l2, att_r2, b2):
    from concourse.bass_utils import run_bass_kernel_spmd

    in_maps, meta = _host_prep(
        x, edge_index, W1, att_l1, att_r1, b1, W2, att_l2, att_r2, b2
    )
    nc = _build_program(meta)
    res = run_bass_kernel_spmd(nc, in_maps, core_ids=list(range(N_CORES)))
    N, SH = meta["N"], meta["SH"]
    OUT_C = meta["OUT_C"]
    full = np.empty((N, OUT_C), np.float32)
    for c in range(N_CORES):
        full[c * SH + meta["perms"][c]] = (
            res.results[c]["out"].astype(np.float32) / S_OUT
        )
    return full



# revision 30
# speedup vs baseline: 1.1052x; 1.0286x over previous
"""Two-layer GAT on 8 Trainium2 NeuronCores.

Strategy (dst-partitioned edge parallelism, degree-sorted blocks):
  - Core c owns nodes [c*SH, (c+1)*SH) for the feature matmul and as edge
    destinations, so the segment softmax over incoming edges is core-local.
  - Per core, dst nodes are in-degree sorted into blocks of 128 (one node
    per SBUF partition); a node's incoming edges lie along the free dim.
  - x ships int8 (global scale folded into W1); the P1 matmul runs f16.
  - Edge gathers use nc.gpsimd.dma_gather (int16 indices). The gather
    table packs 4 nodes per row (row = gpos//4, class = gpos%4) so row ids
    fit in int16; each class is a strided column slice of the table.
    Edge slots are grouped per (block, class-of-src) segment, padded to
    the cross-core max; pad slots gather a sentinel unit whose
    alpha_l = -1000 so exp() -> 0. A host-side greedy balances each
    node's class (pos%4) against its out-edges' destinations, shrinking
    the per-(block,class) max widths ~27%.
  - Layer-1 units are [xl f16 x128 | alpha_l f32 | pad] (512B); layer-2
    units are [h2 f16 x40 | alpha_l2 f32 | pad] (256B).
  - Per gather window the whole slab is processed with fused ops:
    z = alpha_l + armap (per-column alpha_r map), leaky, exp(z-4)
    (shift is softmax-invariant, keeps f16 partials small), payload*ex.
    Each (block,class) segment does one f16 reduce into a per-class
    accumulator bank; cross-class combine + reciprocal are batched per
    group of GB blocks at flush time.
  - The layer-2 projection (W2, att vectors) is fused into the layer-1
    block epilogue (PE transpose + matmul); an 8-core AllGather exchanges
    the packed tables between layers. Output is int16 (x512 scale).
"""

import sys

for _p in ("/opt/trn_rl_repo",):
    if _p not in sys.path:
        sys.path.insert(0, _p)

import numpy as np

try:
    import tempfile

    import jax

    _cc_dir = tempfile.gettempdir() + "/jax_cc_cache"
    jax.config.update("jax_compilation_cache_dir", _cc_dir)
    jax.config.update("jax_persistent_cache_min_entry_size_bytes", -1)
    jax.config.update("jax_persistent_cache_min_compile_time_secs", 0)
except Exception:
    pass

N_CORES = 8
P = 128
GB = 33        # blocks per sweep group
WCOLS = 48     # max gather-window width in slot-columns (128 edges each)
SENT_AL = -1000.0
S_OUT = 512.0  # int16 output scale


# ---------------------------------------------------------------- host prep
def _host_prep(x, edge_index, W1, att_l1, att_r1, b1, W2, att_l2, att_r2, b2):
    x = np.asarray(x, np.float32)
    ei = np.asarray(edge_index).astype(np.int64)
    W1 = np.asarray(W1, np.float32)
    W2 = np.asarray(W2, np.float32)
    att_l1 = np.asarray(att_l1, np.float32)
    att_r1 = np.asarray(att_r1, np.float32)
    att_l2 = np.asarray(att_l2, np.float32)
    att_r2 = np.asarray(att_r2, np.float32)
    b1 = np.asarray(b1, np.float32)
    b2 = np.asarray(b2, np.float32)

    N, IN_C = x.shape
    HID = W1.shape[0]
    OUT_C = W2.shape[0]
    assert N % (N_CORES * 4) == 0
    SH = N // N_CORES
    NBLK = -(-SH // P)
    NROWS = N // 4  # packed table rows
    src, dst = ei[0], ei[1]
    owner = dst // SH

    perms = []
    for c in range(N_CORES):
        m = owner == c
        d0 = dst[m] - c * SH
        deg = np.bincount(d0, minlength=SH)
        perms.append(np.argsort(deg, kind="stable"))

    # --- class balancing: greedily pick each node's table class (pos%4)
    # to flatten per-(dst,class) incoming-edge counts, shrinking the
    # per-(block,class) max widths (= gather padding). Blocks keep their
    # degree-sorted membership; only the within-block lane order changes.
    pos = np.empty(N, np.int64)
    for c in range(N_CORES):
        pos[c * SH + perms[c]] = np.arange(SH)
    outdeg = np.bincount(src, minlength=N)
    indeg = np.bincount(dst, minlength=N)
    fair = np.ceil(indeg / 4.0).astype(np.int32)
    cnt4 = np.zeros((N, 4), np.int32)
    order_e = np.argsort(src, kind="stable")
    ed = dst[order_e]
    starts = np.searchsorted(src[order_e], np.arange(N))
    ends = np.searchsorted(src[order_e], np.arange(N) + 1)
    blk_key = (np.arange(N) // SH) * NBLK + (pos % SH) // P
    bsz = np.bincount(blk_key, minlength=N_CORES * NBLK)
    cap = np.repeat(bsz // 4, 4).reshape(-1, 4).astype(np.int64).copy()
    cls = np.empty(N, np.int64)
    for u in np.argsort(-outdeg, kind="stable"):
        e0, e1 = starts[u], ends[u]
        bk = blk_key[u]
        if e1 > e0:
            vs = ed[e0:e1]
            c4 = cnt4[vs]
            over = np.maximum(c4 + 1 - fair[vs][:, None], 0).sum(axis=0)
            cost = over * 1000.0 + c4.sum(axis=0)
        else:
            cost = np.zeros(4)
        cost = np.where(cap[bk] > 0, cost, np.inf)
        mm = int(np.argmin(cost))
        cls[u] = mm
        cap[bk, mm] -= 1
        if e1 > e0:
            cnt4[vs, mm] += 1
    # rebuild perms: within each block, lane l holds a class l%4 node
    invperms = []
    for c in range(N_CORES):
        newp = np.empty(SH, np.int64)
        cl_loc = cls[c * SH + perms[c]]  # class per old position
        for b in range(NBLK):
            lo = b * P
            hi = min(lo + P, SH)
            blknodes = perms[c][lo:hi]
            blkcls = cl_loc[lo:hi]
            nb = hi - lo
            for mm in range(4):
                sel = blknodes[blkcls == mm]
                newp[lo + mm : lo + nb : 4] = sel
        perms[c] = newp
        inv = np.empty(SH, np.int64)
        inv[newp] = np.arange(SH)
        invperms.append(inv)

    gpos = np.empty(N, np.int64)
    for c in range(N_CORES):
        gpos[c * SH + perms[c]] = c * SH + np.arange(SH)

    # per (block, class) widths, common max across cores
    Wbm = np.zeros((NBLK, 4), np.int64)
    per_core = []
    for c in range(N_CORES):
        m = owner == c
        s_c = src[m]
        d0 = dst[m] - c * SH
        pos = invperms[c][d0]         # dst slot position (block*128+lane)
        g = gpos[s_c]                 # src table position
        cls = (g % 4).astype(np.int64)
        row = g // 4
        blk = pos // P
        lane = pos % P
        cnt = np.zeros((NBLK, 4, P), np.int64)
        np.add.at(cnt, (blk, cls, lane), 1)
        Wbm = np.maximum(Wbm, cnt.max(axis=2))
        per_core.append((row, cls, blk, lane))

    # grid: groups of GB blocks, class-major inside the group
    colstart = np.zeros((NBLK, 4), np.int64)
    windows = []  # (colstart_global, ncols, class, group) per gather call
    col = 0
    b0 = 0
    grp = 0
    while b0 < NBLK:
        b1_ = min(b0 + GB, NBLK)
        for m in range(4):
            wstart = col
            wcols = 0
            for b in range(b0, b1_):
                w = int(Wbm[b, m])
                if wcols + w > WCOLS and wcols > 0:
                    windows.append((wstart, wcols, m, grp))
                    wstart = col
                    wcols = 0
                colstart[b, m] = col
                col += w
                wcols += w
            if wcols > 0:
                windows.append((wstart, wcols, m, grp))
        b0 = b1_
        grp += 1
    totcols = int(col)
    tot_slots = totcols * P
    tot_slots16 = -(-tot_slots // 16) * 16

    # int8-quantize x with a global scale folded into W1 (xl = x_q @ (s*W1).T)
    xscale = max(float(np.abs(x).max()), 1e-8) / 127.0
    w1s = (W1 * xscale).astype(np.float32)
    w1a = np.concatenate(
        [w1s.T, (w1s.T @ att_l1)[:, None], (w1s.T @ att_r1)[:, None]], axis=1
    ).astype(np.float16)
    w2a = np.concatenate(
        [W2.T, (W2.T @ att_l2)[:, None], (W2.T @ att_r2)[:, None]], axis=1
    ).astype(np.float32)
    b1b = np.tile(b1[None, :], (P, 1)).astype(np.float32)
    b2b = np.tile(b2[None, :], (P, 1)).astype(np.float32)
    xq = np.clip(np.round(x / xscale), -127, 127).astype(np.int8)

    in_maps = []
    for c in range(N_CORES):
        row, cls, blk, lane = per_core[c]
        key = (blk * 4 + cls) * P + lane
        order = np.argsort(key, kind="stable")
        ks = key[order]
        rs = row[order]
        cnt2 = np.bincount(ks, minlength=NBLK * 4 * P)
        starts = np.cumsum(cnt2) - cnt2
        w = np.arange(len(ks)) - starts[ks]
        bs = ks // (4 * P)
        ms = (ks // P) % 4
        ls = ks % P
        slot = (colstart[bs, ms] + w) * P + ls
        A = np.full(tot_slots16, NROWS, np.int64)  # sentinel row
        A[slot] = rs
        idx = A.reshape(-1, 16).T.astype(np.int16)  # [16, tot_slots16/16]
        xpt = np.ascontiguousarray(xq[c * SH + perms[c], :].T)
        in_maps.append(
            {
                "xpt": xpt,
                "w1a": w1a,
                "w2a": w2a,
                "b1b": b1b,
                "b2b": b2b,
                "idx": idx,
            }
        )

    meta = dict(
        N=N, SH=SH, NBLK=NBLK, IN_C=IN_C, HID=HID, OUT_C=OUT_C,
        NROWS=NROWS, Wbm=Wbm.tolist(), colstart=colstart.tolist(),
        windows=windows, totcols=totcols, perms=perms,
        idxcols=tot_slots16 // 16,
    )
    return in_maps, meta


# ------------------------------------------------------------- bass program
def _build_program(meta, num_devices=N_CORES):
    from concourse import bacc, mybir, tile
    from concourse.masks import make_identity

    f32 = mybir.dt.float32
    bf16 = mybir.dt.bfloat16
    f16 = mybir.dt.float16
    i16 = mybir.dt.int16
    i8 = mybir.dt.int8
    Alu = mybir.AluOpType
    Act = mybir.ActivationFunctionType
    AxisX = mybir.AxisListType.X

    SH = meta["SH"]
    NBLK = meta["NBLK"]
    IN_C = meta["IN_C"]
    HID = meta["HID"]
    OUT_C = meta["OUT_C"]
    NROWS = meta["NROWS"]
    Wbm = meta["Wbm"]
    colstart = meta["colstart"]
    windows = meta["windows"]
    N = meta["N"]
    idxcols = meta["idxcols"]
    totcols = meta["totcols"]
    KC = IN_C // P
    assert IN_C % P == 0 and HID == P
    SHR = SH // 4  # local packed rows

    U1 = 256       # L1 unit: f16 elems (512B): [xl*128 | a_l f32 | pad]
    U2 = 128       # L2 unit: f16 elems (256B): [h2*40 | a_l2 f32 | pad]
    AL1_F32COL = 64   # f32-view col of a_l within L1 unit
    AL2_F32COL = 20   # f32-view col of a_l2 within L2 unit

    nbs = [min(P, SH - b * P) for b in range(NBLK)]
    maxW = max(1, max(max(r) for r in Wbm))
    max_wcols = max(w for (_, w, _, _) in windows) if windows else 1

    nc = bacc.Bacc(
        "TRN2", target_bir_lowering=False, debug=False, num_devices=num_devices
    )

    xpt = nc.dram_tensor("xpt", [IN_C, SH], i8, kind="ExternalInput")
    w1a = nc.dram_tensor("w1a", [IN_C, HID + 2], f16, kind="ExternalInput")
    w2a = nc.dram_tensor("w2a", [HID, OUT_C + 2], f32, kind="ExternalInput")
    b1b = nc.dram_tensor("b1b", [P, HID], f32, kind="ExternalInput")
    b2b = nc.dram_tensor("b2b", [P, OUT_C], f32, kind="ExternalInput")
    idx = nc.dram_tensor("idx", [16, idxcols], i16, kind="ExternalInput")
    out = nc.dram_tensor("out", [SH, OUT_C], i16, kind="ExternalOutput")

    groups = [list(range(num_devices))]

    with tile.TileContext(nc) as tc:
        with (
            tc.tile_pool(name="dram", bufs=1, space="DRAM") as dpool,
            tc.tile_pool(name="const", bufs=1) as cpool,
            tc.tile_pool(name="psumT", bufs=2, space="PSUM") as psumT,
            tc.tile_pool(name="psum2", bufs=2, space="PSUM") as psum2,
        ):
            xloc = dpool.tile([SHR, 4 * U1], f16)
            xltab = dpool.tile([NROWS + 1, 4 * U1], f16)
            h2loc = dpool.tile([SHR, 4 * U2], f16)
            h2tab = dpool.tile([NROWS + 1, 4 * U2], f16)
            idxrep = dpool.tile([P, idxcols], i16)
            for k in range(8):
                nc.sync.dma_start(
                    out=idxrep[:][k * 16 : (k + 1) * 16, :], in_=idx[:, :]
                )

            ident = cpool.tile([P, P], f32)
            make_identity(nc, ident[:])
            w1a_sb = []
            for k in range(KC):
                t = cpool.tile([P, HID + 2], f16, tag=f"w1a{k}")
                nc.sync.dma_start(out=t[:], in_=w1a[k * P : (k + 1) * P, :])
                w1a_sb.append(t)
            w2a_sb = cpool.tile([P, OUT_C + 2], f32)
            nc.sync.dma_start(out=w2a_sb[:], in_=w2a[:, :])
            b1b_sb = cpool.tile([P, HID], f32)
            nc.sync.dma_start(out=b1b_sb[:], in_=b1b[:, :])
            b2b_sb = cpool.tile([P, OUT_C], f32)
            nc.sync.dma_start(out=b2b_sb[:], in_=b2b[:, :])
            ar1_sb = cpool.tile([P, NBLK], f32)
            nc.vector.memset(ar1_sb[:], 0.0)
            ar2_sb = cpool.tile([P, NBLK], f32)
            nc.vector.memset(ar2_sb[:], 0.0)
            nbias4 = cpool.tile([P, 1], f32)
            nc.vector.memset(nbias4[:], -4.0)

            # sentinel rows (all 4 units): payload=0, a_l=-1000
            s1 = cpool.tile([1, 4 * U1], f16)
            nc.vector.memset(s1[:], 0.0)
            s1f = s1[:].bitcast(f32)
            for m in range(4):
                c0 = m * (U1 // 2) + AL1_F32COL
                nc.vector.memset(s1f[:, c0 : c0 + 1], SENT_AL)
            nc.sync.dma_start(out=xltab[:][NROWS : NROWS + 1, :], in_=s1[:])
            s2 = cpool.tile([1, 4 * U2], f16)
            nc.vector.memset(s2[:], 0.0)
            s2f = s2[:].bitcast(f32)
            for m in range(4):
                c0 = m * (U2 // 2) + AL2_F32COL
                nc.vector.memset(s2f[:, c0 : c0 + 1], SENT_AL)
            nc.sync.dma_start(out=h2tab[:][NROWS : NROWS + 1, :], in_=s2[:])

            # ---------------- P1
            with (
                tc.tile_pool(name="xk", bufs=1) as xkpool,
                tc.tile_pool(name="p1", bufs=3) as p1pool,
                tc.tile_pool(name="psum1", bufs=3, space="PSUM") as psum1,
            ):
                xk = []
                for k in range(KC):
                    t8 = xkpool.tile([P, SH], i8, tag=f"xk8{k}")
                    nc.sync.dma_start(out=t8[:], in_=xpt[k * P : (k + 1) * P, :])
                    t = xkpool.tile([P, SH], f16, tag=f"xk{k}")
                    nc.vector.tensor_copy(t[:], t8[:])
                    xk.append(t)
                xlocflat = xloc[:].rearrange("a b -> (a b)")
                for t in range(NBLK):
                    nb = nbs[t]
                    ps = psum1.tile([P, HID + 2], f32, tag="ps1")
                    for k in range(KC):
                        nc.tensor.matmul(
                            ps[:nb, :],
                            lhsT=xk[k][:, t * P : t * P + nb],
                            rhs=w1a_sb[k][:],
                            start=(k == 0),
                            stop=(k == KC - 1),
                        )
                    unit = p1pool.tile([P, U1], f16, tag="unit")
                    nc.vector.memset(unit[:, HID + 2 : U1], 0.0)
                    nc.vector.tensor_copy(unit[:nb, 0:HID], ps[:nb, 0:HID])
                    uf = unit[:].bitcast(f32)
                    nc.vector.tensor_copy(
                        uf[:nb, AL1_F32COL : AL1_F32COL + 1],
                        ps[:nb, HID : HID + 1],
                    )
                    nc.vector.tensor_copy(
                        ar1_sb[:nb, t : t + 1], ps[:nb, HID + 1 : HID + 2]
                    )
                    # contiguous packed write: local node n -> bf16 elems n*U1
                    dst = xlocflat[t * P * U1 : (t * P + nb) * U1]
                    nc.sync.dma_start(
                        out=dst.rearrange("(a b) -> a b", b=U1), in_=unit[:nb, :]
                    )

            nc.gpsimd.collective_compute(
                "AllGather",
                Alu.bypass,
                replica_groups=groups,
                ins=[xloc[:].opt()],
                outs=[xltab[:][0:NROWS, :].opt()],
            )

            # ---------------- edge phase (shared between layers)
            def edge_phase(tab, UNIT, CF, alcol_f32, ar_sb, bias_sb, tab_f32,
                           finalize):
                gdt = f32 if tab_f32 else f16
                FU = UNIT if tab_f32 else UNIT // 2  # f32-view width
                with (
                    tc.tile_pool(name="gat", bufs=4) as gpool,
                    tc.tile_pool(name="acc", bufs=1) as apool,
                    tc.tile_pool(name="eb", bufs=3) as spool,
                    tc.tile_pool(name="fl", bufs=2) as flpool,
                    tc.tile_pool(name="scl", bufs=2) as sclpool,
                    tc.tile_pool(name="idxp", bufs=4) as ipool,
                    tc.tile_pool(name="arm", bufs=1) as armpool,
                ):
                    # per-column dst bias map (alpha_r of the column's block)
                    armap = armpool.tile([P, totcols], f32)
                    for b in range(NBLK):
                        for mm in range(4):
                            W = Wbm[b][mm]
                            if W == 0:
                                continue
                            s = colstart[b][mm]
                            nc.vector.tensor_copy(
                                armap[:, s : s + W],
                                ar_sb[:, b : b + 1].broadcast_to([P, W]),
                            )
                    # per-(class, block) accumulator bank: one reduce per
                    # segment, cross-class combine + division batched per
                    # group. Double-buffered so the flush of group g doesn't
                    # stall group g+1's windows.
                    def flush_group(g, acc3, accD3):
                        blist = list(range(g * GB, min((g + 1) * GB, NBLK)))
                        ng = len(blist)
                        dsum = flpool.tile([P, GB], f32, tag="dsum")
                        nc.vector.tensor_reduce(
                            out=dsum[:, 0:ng],
                            in_=accD3[:, :, 0:ng].transpose([0, 2, 1]),
                            axis=AxisX, op=Alu.add,
                        )
                        nc.vector.tensor_scalar_max(
                            dsum[:, 0:ng], dsum[:, 0:ng], 1e-16
                        )
                        rden = flpool.tile([P, GB], f32, tag="rden")
                        nc.vector.reciprocal(rden[:, 0:ng], dsum[:, 0:ng])
                        for b in blist:
                            bb = b % GB
                            aT = flpool.tile([P, CF], f32, tag="aTs")
                            nc.vector.tensor_reduce(
                                out=aT[:],
                                in_=acc3[:, :, bb, :].transpose([0, 2, 1]),
                                axis=AxisX, op=Alu.add,
                            )
                            res = flpool.tile([P, CF], f32, tag="res")
                            nc.vector.scalar_tensor_tensor(
                                out=res[:], in0=aT[:],
                                scalar=rden[:, bb : bb + 1],
                                in1=bias_sb[:], op0=Alu.mult, op1=Alu.add,
                            )
                            finalize(b, res)

                    cur_g = None
                    accT = accD = acc3 = accD3 = None
                    for (c0, wc, m, g) in windows:
                        if g != cur_g:
                            if cur_g is not None:
                                flush_group(cur_g, acc3, accD3)
                            accT = apool.tile(
                                [P, 4 * GB * CF], f16, tag="accT"
                            )
                            accD = apool.tile([P, 4 * GB], f32, tag="accD")
                            acc3 = accT[:].rearrange(
                                "p (m b c) -> p m b c", m=4, b=GB
                            )
                            accD3 = accD[:].rearrange(
                                "p (m b) -> p m b", m=4
                            )
                            nc.vector.memset(accT[:], 0.0)
                            nc.vector.memset(accD[:], 0.0)
                            cur_g = g
                        gt = gpool.tile([P, max_wcols * UNIT], gdt, tag="gt")
                        islab = ipool.tile([P, WCOLS * 8], i16, tag="islab")
                        nc.sync.dma_start(
                            out=islab[:, 0 : wc * 8],
                            in_=idxrep[:][:, c0 * 8 : (c0 + wc) * 8],
                        )
                        nidx = wc * P
                        nc.gpsimd.dma_gather(
                            out_ap=gt[:, 0 : wc * UNIT].rearrange(
                                "p (w c) -> p w c", c=UNIT
                            ),
                            in_ap=tab[:][:, m * UNIT : (m + 1) * UNIT],
                            idxs_ap=islab[:, 0 : wc * 8],
                            num_idxs=nidx,
                            num_idxs_reg=nidx,
                            elem_size=UNIT,
                            elem_step=4 * UNIT,
                            single_packet=False,
                        )
                        g3 = gt[:, 0 : wc * UNIT].rearrange(
                            "p (w c) -> p w c", c=UNIT
                        )
                        if tab_f32:
                            g3f = g3
                        else:
                            g3f = gt[:, 0 : wc * UNIT].bitcast(f32).rearrange(
                                "p (w c) -> p w c", c=FU
                            )
                        # fused whole-window: z = a_l + a_r; leaky; exp; msgs
                        alv = g3f[:, :, alcol_f32 : alcol_f32 + 1].squeeze(2)
                        zt = spool.tile([P, WCOLS], f32, tag="z")
                        z = zt[:, 0:wc]
                        nc.vector.tensor_tensor(
                            out=z, in0=alv, in1=armap[:, c0 : c0 + wc],
                            op=Alu.add,
                        )
                        et = spool.tile([P, WCOLS], f32, tag="e")
                        e = et[:, 0:wc]
                        nc.vector.scalar_tensor_tensor(
                            out=e, in0=z, scalar=0.2, in1=z,
                            op0=Alu.mult, op1=Alu.max,
                        )
                        ext = spool.tile([P, WCOLS], gdt, tag="ex")
                        ex = ext[:, 0:wc]
                        nc.scalar.activation(ex, e, Act.Exp, bias=nbias4[:])
                        scl = sclpool.tile([P, max_wcols * CF], gdt, tag="scl")
                        scl3 = scl[:, 0 : wc * CF].rearrange(
                            "p (w c) -> p w c", c=CF
                        )
                        nc.vector.tensor_tensor(
                            out=scl3,
                            in0=g3[:, :, 0:CF],
                            in1=ex.unsqueeze(2).broadcast_to([P, wc, CF]),
                            op=Alu.mult,
                        )
                        for b in range(NBLK):
                            W = Wbm[b][m]
                            s = colstart[b][m]
                            if W == 0 or s < c0 or s >= c0 + wc:
                                continue
                            o = s - c0
                            slot = m * GB + (b % GB)
                            with nc.allow_low_precision(
                                reason="short f16 segment sums; normalized "
                                "by f32 denominators downstream"
                            ):
                                nc.vector.tensor_reduce(
                                    out=accT[:, slot * CF : (slot + 1) * CF],
                                    in_=scl3[:, o : o + W, :].transpose(
                                        [0, 2, 1]
                                    ),
                                    axis=AxisX, op=Alu.add,
                                )
                            nc.vector.tensor_reduce(
                                out=accD[:, slot : slot + 1],
                                in_=ex[:, o : o + W],
                                axis=AxisX, op=Alu.add,
                            )
                    if cur_g is not None:
                        flush_group(cur_g, acc3, accD3)

            # ---------------- L1 finalize: ELU + fused W2 projection
            with tc.tile_pool(name="fin1", bufs=3) as fpool:
                h2locflat = h2loc[:].rearrange("a b -> (a b)")

                def fin1(b, hpre):
                    nb = nbs[b]
                    xm = fpool.tile([P, HID], f32, tag="xm")
                    nc.vector.tensor_scalar_min(xm[:], hpre[:], 0.0)
                    em = fpool.tile([P, HID], f32, tag="em")
                    nc.scalar.activation(em[:], xm[:], Act.Exp)
                    h = fpool.tile([P, HID], f32, tag="h")
                    nc.vector.scalar_tensor_tensor(
                        out=h[:], in0=hpre[:], scalar=0.0, op0=Alu.max,
                        in1=em[:], op1=Alu.add,
                    )
                    nc.vector.tensor_scalar_add(h[:], h[:], -1.0)
                    hT_ps = psumT.tile([P, P], f32, tag="hT")
                    nc.tensor.transpose(hT_ps[:], h[:], ident[:])
                    hT = fpool.tile([P, P], f32, tag="hTs")
                    nc.vector.tensor_copy(hT[:], hT_ps[:])
                    h2ps = psum2.tile([P, OUT_C + 2], f32, tag="h2ps")
                    nc.tensor.matmul(
                        h2ps[:nb, :], lhsT=hT[:, :nb], rhs=w2a_sb[:],
                        start=True, stop=True,
                    )
                    unit = fpool.tile([P, U2], f16, tag="u2")
                    nc.vector.memset(unit[:, OUT_C + 2 : U2], 0.0)
                    nc.vector.tensor_copy(
                        unit[:nb, 0:OUT_C], h2ps[:nb, 0:OUT_C]
                    )
                    uf2 = unit[:].bitcast(f32)
                    nc.vector.tensor_copy(
                        uf2[:nb, AL2_F32COL : AL2_F32COL + 1],
                        h2ps[:nb, OUT_C : OUT_C + 1],
                    )
                    nc.vector.tensor_copy(
                        ar2_sb[:nb, b : b + 1], h2ps[:nb, OUT_C + 1 : OUT_C + 2]
                    )
                    dstf = h2locflat[b * P * U2 : (b * P + nb) * U2]
                    nc.sync.dma_start(
                        out=dstf.rearrange("(a b) -> a b", b=U2),
                        in_=unit[:nb, :],
                    )

                edge_phase(
                    xltab, U1, HID, AL1_F32COL, ar1_sb, b1b_sb, False, fin1
                )

            nc.gpsimd.collective_compute(
                "AllGather",
                Alu.bypass,
                replica_groups=groups,
                ins=[h2loc[:].opt()],
                outs=[h2tab[:][0:NROWS, :].opt()],
            )

            # ---------------- L2 finalize: log_softmax + output
            with tc.tile_pool(name="fin2", bufs=3) as f2pool:

                def fin2(b, logits):
                    nb = nbs[b]
                    nm = f2pool.tile([P, 1], f32, tag="nm")
                    nc.vector.tensor_reduce(
                        out=nm[:], in_=logits[:], axis=AxisX, op=Alu.max,
                        negate=True,
                    )
                    exl = f2pool.tile([P, OUT_C], f32, tag="exl")
                    ssum = f2pool.tile([P, 1], f32, tag="ssum")
                    nc.scalar.activation(
                        exl[:], logits[:], Act.Exp, bias=nm[:],
                        accum_out=ssum[:],
                    )
                    lns = f2pool.tile([P, 1], f32, tag="lns")
                    nc.scalar.activation(lns[:], ssum[:], Act.Ln)
                    cc = f2pool.tile([P, 1], f32, tag="cc")
                    nc.vector.tensor_tensor(
                        out=cc[:], in0=nm[:], in1=lns[:], op=Alu.subtract
                    )
                    fin = f2pool.tile([P, OUT_C], i16, tag="fin")
                    nc.vector.tensor_scalar(
                        out=fin[:], in0=logits[:], scalar1=cc[:],
                        scalar2=S_OUT, op0=Alu.add, op1=Alu.mult,
                    )
                    nc.sync.dma_start(
                        out=out[b * P : b * P + nb, :], in_=fin[:nb, :]
                    )

                edge_phase(h2tab, U2, OUT_C, AL2_F32COL, ar2_sb, b2b_sb, False, fin2)

    nc.compile()
    return nc


# ------------------------------------------------------------------- driver
def kernel(x, edge_index, W1, att_l1, att_r1, b1, W2, att_l2, att_r2, b2):
    from concourse.bass_utils import run_bass_kernel_spmd

    in_maps, meta = _host_prep(
        x, edge_index, W1, att_l1, att_r1, b1, W2, att_l2, att_r2, b2
    )
    nc = _build_program(meta)
    res = run_bass_kernel_spmd(nc, in_maps, core_ids=list(range(N_CORES)))
    N, SH = meta["N"], meta["SH"]
    OUT_C = meta["OUT_C"]
    full = np.empty((N, OUT_C), np.float32)
    for c in range(N_CORES):
        full[c * SH + meta["perms"][c]] = (
            res.results[c]["out"].astype(np.float32) / S_OUT
        )
    return full

